# revision 19
# baseline (speedup 1.0000x reference)
"""Causal self-attention (B=2, S=2048, E=1024, H=16) on 8 trn2 cores.

Sharding: batch x head -- core c handles batch c//4 and the 4 heads
starting at (c%4)*4. Each core runs QKV projection for its heads,
causal attention, and its slice of the output projection (row-split
c_proj); the host sums the 4 partial projections per batch.

Single fused software-pipelined loop: attention for q-tile qj runs
with the QKV projection of tile qj+1 and the output projection of
tile qj-1 interleaved as PE fillers, so the tensor engine never
idles (and never drops out of its fast p-state). DRAM inputs are
bf16 (halves DMA), internal attention tensors fp16, projections
bf16; scores are computed transposed so every matmul streams 512
moving columns. Row-sums ride as a ones-column appended to V;
softmax normalization is broadcast via a tiny [2,128]-stationary
matmul per head pair.
"""

import os
import sys

import numpy as np

_DIR = os.path.dirname(os.path.abspath(__file__))
for _p in (_DIR,):
    if _p not in sys.path:
        sys.path.insert(0, _p)

import concourse.bass as bass
import concourse.mybir as mybir
from concourse import tile
from concourse.vector_clock import ScopedClock, VectorClock

F32 = mybir.dt.float32
F32R = mybir.dt.float32r
BF16 = mybir.dt.bfloat16
F16 = mybir.dt.float16
U16 = mybir.dt.uint16

B, S, E, H, D = 2, 2048, 1024, 16, 64
HPC = 4          # heads per core
N_CORES = 8
QT = 512         # q tile (moving dim)
KC = 128         # k chunk (contraction tile)
EC = E // 128    # 8 contraction chunks over the embedding dim
NQ = S // QT     # 4 q tiles
NST = S // 128   # 16 s tiles of 128


class SplitDrainTileContext(tile.TileContext):
    """Kernel-tail drain with its sem waits split one per instruction.

    The walrus build here rejects instructions carrying more sync waits
    than their ISA struct encodes; TileContext hangs one wait per live
    proc on a single Drain. Sequential single-wait drains on the sync
    engine give the same guarantee.
    """

    def _drain_and_barrier(self, tick_clock, wait_clock):
        gc = list(tick_clock.global_clock)
        n = len(gc)
        for i, t in enumerate(gc):
            if t:
                vc = VectorClock([t if j == i else 0 for j in range(n)])
                inst = self.nc.sync.drain()
                wait_clock.add_sem_waits(inst.ins, ScopedClock({None: vc}))
        self.nc.all_engine_barrier()
        assert self.sems is not None
        popped = self.nc._tile_sem_poison_stack.pop()
        assert popped is self._sem_poison
        self.nc.clear_and_free_semaphores(list(self.sems.allocated().values()))
        self.nc.all_engine_barrier()


# ---------------------------------------------------------------- BIR fix

_CAPS = {"EventSemaphore": 2}
_DEFAULT_CAP = 1
_counter = [0]


def _split_bir_waits(bir):
    """Move excess sync waits onto EventSemaphores inserted just before
    the overloaded instruction (same engine => same program order)."""
    n = 0
    for fn in bir.get("functions", []):
        for bb in fn.get("blocks", []):
            out = []
            for inst in bb.get("instructions", []):
                si = inst.get("sync_info")
                waits = si.get("on_wait") if si else None
                cap = _CAPS.get(inst.get("opcode"), _DEFAULT_CAP)
                if waits and len(waits) > cap:
                    excess, keep = waits[:-cap], waits[-cap:]
                    for i in range(0, len(excess), 2):
                        _counter[0] += 1
                        out.append({
                            "debug": inst.get("debug", 0),
                            "engine": inst["engine"],
                            "ins": [], "outs": [],
                            "name": f"antsplitw-{_counter[0]}",
                            "opcode": "EventSemaphore",
                            "sync_info": {"on_update": [],
                                          "on_wait": excess[i:i + 2]},
                        })
                        n += 1
                    si["on_wait"] = keep
                out.append(inst)
            bb["instructions"] = out
    return n


def _install_bir_fix():
    import json
    import concourse.bass2jax as bass2jax
    from concourse.bass_utils import compile_bir_kernel as orig
    if getattr(bass2jax.compile_bir_kernel, "_ant_split", False):
        return

    def wrapped(ant_bir_str, *args, **kwargs):
        bir = json.loads(ant_bir_str)
        if _split_bir_waits(bir):
            ant_bir_str = json.dumps(bir).encode()
        return orig(ant_bir_str, *args, **kwargs)

    wrapped._ant_split = True
    bass2jax.compile_bir_kernel = wrapped


# ---------------------------------------------------------------- device

def build():
    nc = bass.Bass("TRN2", target_bir_lowering=False, debug=False)
    xT_d = nc.dram_tensor("xT", [E, S], BF16, kind="ExternalInput").ap()
    wqk_d = nc.dram_tensor("wqk", [E, 2 * HPC * D], BF16, kind="ExternalInput").ap()
    wv_d = nc.dram_tensor("wv", [E, HPC * D], BF16, kind="ExternalInput").ap()
    wp_d = nc.dram_tensor("wproj", [HPC * D, E], BF16, kind="ExternalInput").ap()
    bmat_d = nc.dram_tensor("bmat", [2, 128], F32R, kind="ExternalInput").ap()
    y_d = nc.dram_tensor("y", [S, E], F32, kind="ExternalOutput").ap()

    with SplitDrainTileContext(nc) as tc:
        with (
            tc.tile_pool(name="persist", bufs=1) as persist,
            tc.tile_pool(name="pss", bufs=2, space="PSUM") as pss,
            tc.tile_pool(name="psav", bufs=2, space="PSUM") as psav,
            tc.tile_pool(name="psq", bufs=2, space="PSUM") as psq,
            tc.tile_pool(name="ptp", bufs=6) as ptp,
            tc.tile_pool(name="nrm", bufs=4) as nrm,
            tc.tile_pool(name="bcs2", bufs=2) as bcs2,
            tc.tile_pool(name="pout", bufs=3) as pout,
        ):
            xT_sb = persist.tile([128, EC, S], BF16)
            wqk_sb = persist.tile([128, EC, 512], BF16)
            wv_sb = persist.tile([128, EC, 256], BF16)
            wp_sb = persist.tile([128, 2, E], BF16)
            qT_sb = persist.tile([128, 2, S], F16)    # heads 01 | 23 stacked
            kTpad = persist.tile([128, HPC, S], F16)  # per head, half rows zero
            vaug = persist.tile([128, NST, HPC, D + 1], F16)
            # un-normalized y^T, head pairs stacked in partition halves
            yTun = persist.tile([128, NQ * 2, QT], F32)
            yT = persist.tile([128, 2, S], BF16)       # normalized, proj lhsT
            bmat = persist.tile([2, 128], F32R)

            nc.vector.memset(vaug[:, :, :, D:D + 1].bitcast(U16), 15360)  # fp16 1.0
            for h in range(HPC):
                dead = slice(64, 128) if h % 2 == 0 else slice(0, 64)
                nc.vector.memset(kTpad[dead, h, :].bitcast(U16), 0)

            # ---- all input DMAs, first-needed first, 3 queues ----
            def xt_piece(eng, ec, q4):
                eng.dma_start(xT_sb[:, ec, q4 * 512:(q4 + 1) * 512],
                              xT_d[ec * 128:(ec + 1) * 128, q4 * 512:(q4 + 1) * 512])
            # wqk/xT(q0) alternate across two queues so the ec-outer
            # qkv(0) loop below gets its (wqk, xT) pairs in arrival order
            for ec in range(EC):
                if ec % 2 == 0:
                    nc.sync.dma_start(wqk_sb[:, ec, :],
                                      wqk_d[ec * 128:(ec + 1) * 128, :])
                    xt_piece(nc.scalar, ec, 0)
                else:
                    nc.scalar.dma_start(wqk_sb[:, ec, :],
                                        wqk_d[ec * 128:(ec + 1) * 128, :])
                    xt_piece(nc.sync, ec, 0)
                nc.gpsimd.dma_start(wv_sb[:, ec, :], wv_d[ec * 128:(ec + 1) * 128, :])
            for ec in range(EC):
                nc.sync.dma_start(xT_sb[:, ec, 512:1024],
                                  xT_d[ec * 128:(ec + 1) * 128, 512:1024])
                xt_piece(nc.scalar, ec, 2)
                xt_piece(nc.gpsimd, ec, 3)
            for ci in range(2):
                nc.gpsimd.dma_start(wp_sb[:, ci, :], wp_d[ci * 128:(ci + 1) * 128, :])
            nc.gpsimd.dma_start(bmat[:], bmat_d[:])

            # ---- emission thunks ----

            def qkv_thunks(q4, qk_done=False):
                """QKV projection for q-tile q4 as a list of single-matmul
                thunks (plus close/copy tails on vector)."""
                sslc = slice(q4 * QT, (q4 + 1) * QT)
                thunks = []
                state = {}

                def qk_mm(rt, ec):
                    def run():
                        if ec == 0:
                            state[rt] = psq.tile([128, QT], F32, name="psqk", tag="mix")
                        nc.tensor.matmul(
                            state[rt][:],
                            wqk_sb[:, ec, rt * 128:(rt + 1) * 128],
                            xT_sb[:, ec, sslc],
                            start=(ec == 0), stop=(ec == EC - 1))
                        if ec == EC - 1:
                            ps = state.pop(rt)
                            if rt < 2:
                                nc.vector.tensor_copy(out=qT_sb[:, rt, sslc],
                                                      in_=ps[:])
                            else:
                                nc.vector.tensor_copy(
                                    out=kTpad[0:64, 2 * (rt - 2), sslc],
                                    in_=ps[0:64, :])
                                nc.vector.tensor_copy(
                                    out=kTpad[64:128, 2 * (rt - 2) + 1, sslc],
                                    in_=ps[64:128, :])
                    return run

                def v_mm(st2, ec):
                    def run():
                        if ec == 0:
                            state['v'] = psq.tile([128, 256], F32, name="psv", tag="mix")
                        nc.tensor.matmul(
                            state['v'][:],
                            xT_sb[:, ec, st2 * 128:(st2 + 1) * 128],
                            wv_sb[:, ec, :],
                            start=(ec == 0), stop=(ec == EC - 1))
                        if ec == EC - 1:
                            ps = state.pop('v')
                            nc.vector.tensor_copy(
                                out=vaug[:, st2, :, 0:D],
                                in_=ps[:, :].rearrange("p (h d) -> p h d", h=HPC))
                    return run

                for rt in range(4):
                    for ec in range(EC):
                        thunks.append(qk_mm(rt, ec))
                for st2 in range(4 * q4, 4 * q4 + 4):
                    for ec in range(EC):
                        thunks.append(v_mm(st2, ec))
                return thunks[32:] if qk_done else thunks

            def recip_half(rs_pair):
                """1/rowsum for one head pair -> [2, QT] f32r tile."""
                rs2 = nrm.tile([2, QT], F32, tag="rs2")
                nc.sync.dma_start(rs2[:, :], rs_pair[0:1, 0:2, :])
                rt2 = nrm.tile([2, QT], F32R, tag="rt2")
                with nc.allow_low_precision(reason="f32r recip"):
                    nc.vector.reciprocal(rt2[:, :], rs2[:, :])
                return rt2

            def proj_thunks(qj, rt2s):
                """bc broadcast + normalization + output projection for
                q-tile qj, as single-matmul thunks."""
                qslc = slice(qj * QT, (qj + 1) * QT)
                thunks = []

                def bc_norm(half):
                    def run():
                        ps = psq.tile([128, QT], F32, name="psbc", tag="mix")
                        nc.tensor.matmul(ps[:], bmat[:, :], rt2s[half][:, :],
                                         start=True, stop=True)
                        bc_sb = bcs2.tile([128, QT], F32R, name="bc_sb")
                        nc.vector.tensor_copy(out=bc_sb[:], in_=ps[:])
                        with nc.allow_low_precision(reason="proj lhsT"):
                            nc.vector.tensor_mul(
                                yT[:, half, qslc],
                                yTun[:, qj * 2 + half, :],
                                bc_sb[:, :])
                    return run

                state = {}

                def pp_mm(qt, eo, ci):
                    def run():
                        if ci == 0:
                            state[(qt, eo)] = psq.tile([128, QT], F32, name="pspp",
                                                       tag="mix")
                        pp = state[(qt, eo)]
                        nc.tensor.matmul(
                            pp[:],
                            yT[:, ci, qt * 128:(qt + 1) * 128],
                            wp_sb[:, ci, eo * 512:(eo + 1) * 512],
                            start=(ci == 0), stop=(ci == 1))
                        if ci == 1:
                            pp = state.pop((qt, eo))
                            po_t = pout.tile([128, 512], F32, name="po_t")
                            nc.vector.tensor_copy(out=po_t[:], in_=pp[:])
                            nc.sync.dma_start(
                                y_d[qt * 128:(qt + 1) * 128,
                                    eo * 512:(eo + 1) * 512],
                                po_t[:])
                    return run

                thunks.append(bc_norm(0))
                thunks.append(bc_norm(1))
                for qt in range(qj * 4, (qj + 1) * 4):
                    for eo in range(E // 512):
                        for ci in range(2):
                            thunks.append(pp_mm(qt, eo, ci))
                return thunks

            def attention(qj, fillers):
                """Causal attention for q-tile qj; pops filler thunks
                between score and AV matmul pairs. Returns rt2 tiles."""
                rt2s = []
                nkc = (qj + 1) * QT // KC
                npairs_total = HPC * (nkc // 2)
                pair_idx = 0
                qslc = slice(qj * QT, (qj + 1) * QT)
                rs_pair = None
                for h in range(HPC):
                    if h % 2 == 0:
                        rs_pair = nrm.tile([1, 2, QT], F32, tag="rsp")
                    qT_ap = qT_sb[:, h // 2, qslc]
                    av = psav.tile([65, QT], F32, name="av")
                    for pr in range(nkc // 2):
                        ps = pss.tile([128, 2, QT], F32, name="ps")
                        offs = [max(0, (2 * pr + j) * KC - qj * QT)
                                for j in range(2)]
                        for j in range(2):
                            kc = 2 * pr + j
                            o = offs[j]
                            nc.tensor.matmul(
                                ps[:, j, o:QT],
                                kTpad[:, h, kc * KC:(kc + 1) * KC],
                                qT_ap[:, o:QT],
                                start=True, stop=True)
                        pt = ptp.tile([128, 2, QT], F16, name="pt")
                        o0 = offs[0]
                        nc.scalar.activation(
                            pt[:, :, o0:QT], ps[:, :, o0:QT],
                            mybir.ActivationFunctionType.Exp,
                            scale=0.125)
                        for j in range(2):
                            kc = 2 * pr + j
                            if kc * KC >= qj * QT:
                                # mask only the 128-wide diagonal band
                                o = offs[j]
                                w = min(KC, QT - o)
                                nc.gpsimd.affine_select(
                                    out=pt[:, j, o:o + w],
                                    in_=pt[:, j, o:o + w],
                                    compare_op=mybir.AluOpType.is_ge,
                                    fill=0.0, base=qj * QT + o - kc * KC,
                                    pattern=[[1, w]],
                                    channel_multiplier=-1)
                        # filler PE work sits in the exp-latency window
                        pair_idx += 1
                        nf = ((len(fillers) * pair_idx) // npairs_total
                              - (len(fillers) * (pair_idx - 1)) // npairs_total)
                        for _ in range(nf):
                            fillers.pop(0)()
                        for j in range(2):
                            kc = 2 * pr + j
                            o = offs[j]
                            nc.tensor.matmul(av[:, o:QT],
                                             vaug[:, kc, h, :],
                                             pt[:, j, o:QT],
                                             start=(kc == 0),
                                             stop=(kc == nkc - 1))
                    po = 64 * (h % 2)
                    nc.vector.tensor_copy(
                        out=yTun[po:po + 64, qj * 2 + h // 2, :],
                        in_=av[0:64, :])
                    nc.vector.tensor_copy(
                        out=rs_pair[0:1, h % 2, :], in_=av[64:65, :])
                    if h % 2 == 1:
                        rt2s.append(recip_half(rs_pair))
                # any leftover fillers (shouldn't happen, but be safe)
                while fillers:
                    fillers.pop(0)()
                return rt2s

            # ---- fused pipeline ----
            # qkv(0) q/k: ec-outer on the (still idle) pss pair tiles so
            # the PE starts on the first-arriving DMA pieces
            psA = pss.tile([128, 2, QT], F32, name="ps")
            psB = pss.tile([128, 2, QT], F32, name="ps")
            for ec in range(EC):
                for rt in range(4):
                    tgt = psA if rt < 2 else psB
                    nc.tensor.matmul(tgt[:, rt % 2, :],
                                     wqk_sb[:, ec, rt * 128:(rt + 1) * 128],
                                     xT_sb[:, ec, 0:QT],
                                     start=(ec == 0), stop=(ec == EC - 1))
            nc.vector.tensor_copy(out=qT_sb[:, :, 0:QT], in_=psA[:, :, :])
            for pr2 in range(2):
                nc.vector.tensor_copy(out=kTpad[0:64, 2 * pr2, 0:QT],
                                      in_=psB[0:64, pr2, :])
                nc.vector.tensor_copy(out=kTpad[64:128, 2 * pr2 + 1, 0:QT],
                                      in_=psB[64:128, pr2, :])
            for t in qkv_thunks(0, qk_done=True):
                t()
            # fillers: attn qj <- qkv(qj+1); proj(0) -> attn1;
            # proj(1)+proj(2) -> attn3 (attn3 is ACT-limited and needs
            # the most PE filler); proj(3) -> tail
            rt2s = [None] * NQ
            rt2s[0] = attention(0, qkv_thunks(1))
            rt2s[1] = attention(1, qkv_thunks(2) + proj_thunks(0, rt2s[0]))
            rt2s[2] = attention(2, qkv_thunks(3))
            rt2s[3] = attention(3, proj_thunks(1, rt2s[1])
                                + proj_thunks(2, rt2s[2]))
            for t in proj_thunks(3, rt2s[3]):
                t()
    return nc


# ---------------------------------------------------------------- host

_NC_CACHE = []


def _get_nc():
    if not _NC_CACHE:
        _install_bir_fix()
        _NC_CACHE.append(build())
    return _NC_CACHE[0]


def make_in_maps(x, w_attn, w_proj):
    import ml_dtypes
    bf16 = ml_dtypes.bfloat16
    bmat = np.zeros((2, 128), np.float32)
    bmat[0, 0:64] = 1.0
    bmat[1, 64:128] = 1.0
    in_maps = []
    for c in range(N_CORES):
        b, h0 = c // 4, (c % 4) * HPC
        wq = w_attn[:, h0 * D:(h0 + HPC) * D]
        wk = w_attn[:, E + h0 * D:E + (h0 + HPC) * D]
        wv = w_attn[:, 2 * E + h0 * D:2 * E + (h0 + HPC) * D]
        in_maps.append({
            "xT": np.ascontiguousarray(x[b].T).astype(bf16),
            "wqk": np.ascontiguousarray(
                np.concatenate([wq, wk], axis=1)).astype(bf16),
            "wv": np.ascontiguousarray(wv).astype(bf16),
            "wproj": np.ascontiguousarray(
                w_proj[h0 * D:(h0 + HPC) * D, :]).astype(bf16),
            "bmat": bmat,
        })
    return in_maps


def run(x, w_attn, w_proj, trace=False, tmpdir=None):
    from concourse.bass_utils import run_bass_kernel_spmd
    nc = _get_nc()
    res = run_bass_kernel_spmd(nc, make_in_maps(x, w_attn, w_proj),
                               list(range(N_CORES)), trace=trace, tmpdir=tmpdir)
    y = np.zeros((B, S, E), np.float32)
    for c in range(N_CORES):
        y[c // 4] += np.asarray(res.results[c]["y"], np.float32)
    return y, res


def kernel(x, w_attn, w_proj):
    y, _ = run(np.asarray(x, np.float32), np.asarray(w_attn, np.float32),
               np.asarray(w_proj, np.float32))
    return y


# revision 20
# speedup vs baseline: 1.0607x; 1.0607x over previous
"""Causal self-attention (B=2, S=2048, E=1024, H=16) on 8 trn2 cores.

Sharding: batch x head -- core c handles batch c//4 and the 4 heads
starting at (c%4)*4. Each core runs QKV projection for its heads,
causal attention, and its slice of the output projection (row-split
c_proj); the host sums the 4 partial projections per batch.

Single fused software-pipelined loop: attention for q-tile qj runs
with the QKV projection of tile qj+1 and the output projection of
tile qj-1 interleaved as PE fillers, so the tensor engine never
idles (and never drops out of its fast p-state). DRAM inputs are
bf16 (halves DMA), internal attention tensors fp16, projections
bf16; scores are computed transposed so every matmul streams 512
moving columns. Row-sums ride as a ones-column appended to V;
softmax normalization is broadcast via a tiny [2,128]-stationary
matmul per head pair.
"""

import os
import sys

import numpy as np

_DIR = os.path.dirname(os.path.abspath(__file__))
for _p in (_DIR,):
    if _p not in sys.path:
        sys.path.insert(0, _p)

import concourse.bass as bass
import concourse.mybir as mybir
from concourse import tile
from concourse.vector_clock import ScopedClock, VectorClock

F32 = mybir.dt.float32
F32R = mybir.dt.float32r
BF16 = mybir.dt.bfloat16
F16 = mybir.dt.float16
U16 = mybir.dt.uint16

B, S, E, H, D = 2, 2048, 1024, 16, 64
HPC = 4          # heads per core
N_CORES = 8
QT = 512         # q tile (moving dim)
KC = 128         # k chunk (contraction tile)
EC = E // 128    # 8 contraction chunks over the embedding dim
NQ = S // QT     # 4 q tiles
NST = S // 128   # 16 s tiles of 128


class SplitDrainTileContext(tile.TileContext):
    """Kernel-tail drain with its sem waits split one per instruction.

    The walrus build here rejects instructions carrying more sync waits
    than their ISA struct encodes; TileContext hangs one wait per live
    proc on a single Drain. Sequential single-wait drains on the sync
    engine give the same guarantee.
    """

    def _drain_and_barrier(self, tick_clock, wait_clock):
        gc = list(tick_clock.global_clock)
        n = len(gc)
        for i, t in enumerate(gc):
            if t:
                vc = VectorClock([t if j == i else 0 for j in range(n)])
                inst = self.nc.sync.drain()
                wait_clock.add_sem_waits(inst.ins, ScopedClock({None: vc}))
        self.nc.all_engine_barrier()
        assert self.sems is not None
        popped = self.nc._tile_sem_poison_stack.pop()
        assert popped is self._sem_poison
        self.nc.clear_and_free_semaphores(list(self.sems.allocated().values()))
        self.nc.all_engine_barrier()


# ---------------------------------------------------------------- BIR fix

_CAPS = {"EventSemaphore": 2}
_DEFAULT_CAP = 1
_counter = [0]


def _split_bir_waits(bir):
    """Move excess sync waits onto EventSemaphores inserted just before
    the overloaded instruction (same engine => same program order)."""
    n = 0
    for fn in bir.get("functions", []):
        for bb in fn.get("blocks", []):
            out = []
            for inst in bb.get("instructions", []):
                si = inst.get("sync_info")
                waits = si.get("on_wait") if si else None
                cap = _CAPS.get(inst.get("opcode"), _DEFAULT_CAP)
                if waits and len(waits) > cap:
                    excess, keep = waits[:-cap], waits[-cap:]
                    for i in range(0, len(excess), 2):
                        _counter[0] += 1
                        out.append({
                            "debug": inst.get("debug", 0),
                            "engine": inst["engine"],
                            "ins": [], "outs": [],
                            "name": f"antsplitw-{_counter[0]}",
                            "opcode": "EventSemaphore",
                            "sync_info": {"on_update": [],
                                          "on_wait": excess[i:i + 2]},
                        })
                        n += 1
                    si["on_wait"] = keep
                out.append(inst)
            bb["instructions"] = out
    return n


def _install_bir_fix():
    import json
    import concourse.bass2jax as bass2jax
    from concourse.bass_utils import compile_bir_kernel as orig
    if getattr(bass2jax.compile_bir_kernel, "_ant_split", False):
        return

    def wrapped(ant_bir_str, *args, **kwargs):
        bir = json.loads(ant_bir_str)
        if _split_bir_waits(bir):
            ant_bir_str = json.dumps(bir).encode()
        return orig(ant_bir_str, *args, **kwargs)

    wrapped._ant_split = True
    bass2jax.compile_bir_kernel = wrapped


# ---------------------------------------------------------------- device

def build():
    nc = bass.Bass("TRN2", target_bir_lowering=False, debug=False)
    xT_d = nc.dram_tensor("xT", [E, S], BF16, kind="ExternalInput").ap()
    wqk_d = nc.dram_tensor("wqk", [E, 2 * HPC * D], BF16, kind="ExternalInput").ap()
    wv_d = nc.dram_tensor("wv", [E, HPC * D], BF16, kind="ExternalInput").ap()
    wp_d = nc.dram_tensor("wproj", [HPC * D, E], BF16, kind="ExternalInput").ap()
    bmat_d = nc.dram_tensor("bmat", [2, 128], F32R, kind="ExternalInput").ap()
    y_d = nc.dram_tensor("y", [S, E], F32, kind="ExternalOutput").ap()

    with SplitDrainTileContext(nc) as tc:
        with (
            tc.tile_pool(name="persist", bufs=1) as persist,
            tc.tile_pool(name="pss", bufs=2, space="PSUM") as pss,
            tc.tile_pool(name="psav", bufs=2, space="PSUM") as psav,
            tc.tile_pool(name="psq", bufs=2, space="PSUM") as psq,
            tc.tile_pool(name="ptp", bufs=6) as ptp,
            tc.tile_pool(name="nrm", bufs=4) as nrm,
            tc.tile_pool(name="bcs2", bufs=2) as bcs2,
            tc.tile_pool(name="pout", bufs=3) as pout,
        ):
            xT_sb = persist.tile([128, EC, S], BF16)
            wqk_sb = persist.tile([128, EC, 512], BF16)
            wv_sb = persist.tile([128, EC, 256], BF16)
            wp_sb = persist.tile([128, 2, E], BF16)
            qT_sb = persist.tile([128, 2, S], F16)    # heads 01 | 23 stacked
            kTpad = persist.tile([128, HPC, S], F16)  # per head, half rows zero
            vaug = persist.tile([128, NST, HPC, D + 1], F16)
            # un-normalized y^T, head pairs stacked in partition halves
            yTun = persist.tile([128, NQ * 2, QT], F32)
            yT = persist.tile([128, 2, S], BF16)       # normalized, proj lhsT
            bmat = persist.tile([2, 128], F32R)

            nc.vector.memset(vaug[:, :, :, D:D + 1].bitcast(U16), 15360)  # fp16 1.0
            for h in range(HPC):
                dead = slice(64, 128) if h % 2 == 0 else slice(0, 64)
                nc.vector.memset(kTpad[dead, h, :].bitcast(U16), 0)

            # ---- all input DMAs, first-needed first, 3 queues ----
            def xt_piece(eng, ec, q4):
                eng.dma_start(xT_sb[:, ec, q4 * 512:(q4 + 1) * 512],
                              xT_d[ec * 128:(ec + 1) * 128, q4 * 512:(q4 + 1) * 512])
            # wqk/xT(q0) alternate across two queues so the ec-outer
            # qkv(0) loop below gets its (wqk, xT) pairs in arrival order
            for ec in range(EC):
                if ec % 2 == 0:
                    nc.sync.dma_start(wqk_sb[:, ec, :],
                                      wqk_d[ec * 128:(ec + 1) * 128, :])
                    xt_piece(nc.scalar, ec, 0)
                else:
                    nc.scalar.dma_start(wqk_sb[:, ec, :],
                                        wqk_d[ec * 128:(ec + 1) * 128, :])
                    xt_piece(nc.sync, ec, 0)
                nc.gpsimd.dma_start(wv_sb[:, ec, :], wv_d[ec * 128:(ec + 1) * 128, :])
            for ec in range(EC):
                nc.sync.dma_start(xT_sb[:, ec, 512:1024],
                                  xT_d[ec * 128:(ec + 1) * 128, 512:1024])
                xt_piece(nc.scalar, ec, 2)
                xt_piece(nc.gpsimd, ec, 3)
            for ci in range(2):
                nc.gpsimd.dma_start(wp_sb[:, ci, :], wp_d[ci * 128:(ci + 1) * 128, :])
            nc.gpsimd.dma_start(bmat[:], bmat_d[:])

            # ---- emission thunks ----

            def qkv_thunks(q4, qk_done=False):
                """QKV projection for q-tile q4 as a list of single-matmul
                thunks (plus close/copy tails on vector)."""
                sslc = slice(q4 * QT, (q4 + 1) * QT)
                thunks = []
                state = {}

                def qk_mm(rt, ec):
                    def run():
                        if ec == 0:
                            state[rt] = psq.tile([128, QT], F32, name="psqk", tag="mix")
                        nc.tensor.matmul(
                            state[rt][:],
                            wqk_sb[:, ec, rt * 128:(rt + 1) * 128],
                            xT_sb[:, ec, sslc],
                            start=(ec == 0), stop=(ec == EC - 1))
                        if ec == EC - 1:
                            ps = state.pop(rt)
                            if rt < 2:
                                nc.vector.tensor_copy(out=qT_sb[:, rt, sslc],
                                                      in_=ps[:])
                            else:
                                nc.vector.tensor_copy(
                                    out=kTpad[0:64, 2 * (rt - 2), sslc],
                                    in_=ps[0:64, :])
                                nc.vector.tensor_copy(
                                    out=kTpad[64:128, 2 * (rt - 2) + 1, sslc],
                                    in_=ps[64:128, :])
                    return run

                def v_mm(st2, ec):
                    def run():
                        if ec == 0:
                            state['v'] = psq.tile([128, 256], F32, name="psv", tag="mix")
                        nc.tensor.matmul(
                            state['v'][:],
                            xT_sb[:, ec, st2 * 128:(st2 + 1) * 128],
                            wv_sb[:, ec, :],
                            start=(ec == 0), stop=(ec == EC - 1))
                        if ec == EC - 1:
                            ps = state.pop('v')
                            nc.vector.tensor_copy(
                                out=vaug[:, st2, :, 0:D],
                                in_=ps[:, :].rearrange("p (h d) -> p h d", h=HPC))
                    return run

                for rt in range(4):
                    for ec in range(EC):
                        thunks.append(qk_mm(rt, ec))
                for st2 in range(4 * q4, 4 * q4 + 4):
                    for ec in range(EC):
                        thunks.append(v_mm(st2, ec))
                return thunks[32:] if qk_done else thunks

            def recip_half(rs_pair):
                """1/rowsum for one head pair -> [2, QT] f32r tile."""
                rs2 = nrm.tile([2, QT], F32, tag="rs2")
                nc.sync.dma_start(rs2[:, :], rs_pair[0:1, 0:2, :])
                lg = nrm.tile([2, QT], F32, tag="lg")
                nc.scalar.activation(lg[:, :], rs2[:, :],
                                     mybir.ActivationFunctionType.Ln)
                rt2 = nrm.tile([2, QT], F32R, tag="rt2")
                # exp(-ln(x)) = 1/x; Ln and Exp share one ACT table set
                nc.scalar.activation(rt2[:, :], lg[:, :],
                                     mybir.ActivationFunctionType.Exp,
                                     scale=-1.0)
                return rt2

            def proj_thunks(qj, rt2s):
                """bc broadcast + normalization + output projection for
                q-tile qj, as single-matmul thunks."""
                qslc = slice(qj * QT, (qj + 1) * QT)
                thunks = []

                def bc_norm(half):
                    def run():
                        ps = psq.tile([128, QT], F32, name="psbc", tag="mix")
                        nc.tensor.matmul(ps[:], bmat[:, :], rt2s[half][:, :],
                                         start=True, stop=True)
                        bc_sb = bcs2.tile([128, QT], F32R, name="bc_sb")
                        nc.vector.tensor_copy(out=bc_sb[:], in_=ps[:])
                        with nc.allow_low_precision(reason="proj lhsT"):
                            nc.vector.tensor_mul(
                                yT[:, half, qslc],
                                yTun[:, qj * 2 + half, :],
                                bc_sb[:, :])
                    return run

                state = {}

                def pp_mm(qt, eo, ci):
                    def run():
                        if ci == 0:
                            state[(qt, eo)] = psq.tile([128, QT], F32, name="pspp",
                                                       tag="mix")
                        pp = state[(qt, eo)]
                        nc.tensor.matmul(
                            pp[:],
                            yT[:, ci, qt * 128:(qt + 1) * 128],
                            wp_sb[:, ci, eo * 512:(eo + 1) * 512],
                            start=(ci == 0), stop=(ci == 1))
                        if ci == 1:
                            pp = state.pop((qt, eo))
                            po_t = pout.tile([128, 512], F32, name="po_t")
                            nc.vector.tensor_copy(out=po_t[:], in_=pp[:])
                            nc.sync.dma_start(
                                y_d[qt * 128:(qt + 1) * 128,
                                    eo * 512:(eo + 1) * 512],
                                po_t[:])
                    return run

                thunks.append(bc_norm(0))
                thunks.append(bc_norm(1))
                for qt in range(qj * 4, (qj + 1) * 4):
                    for eo in range(E // 512):
                        for ci in range(2):
                            thunks.append(pp_mm(qt, eo, ci))
                return thunks

            def attention(qj, fillers):
                """Causal attention for q-tile qj; pops filler thunks
                between score and AV matmul pairs. Returns rt2 tiles."""
                rt2s = []
                nkc = (qj + 1) * QT // KC
                npairs_total = HPC * (nkc // 2)
                pair_idx = 0
                qslc = slice(qj * QT, (qj + 1) * QT)
                rs_pair = None
                for h in range(HPC):
                    if h % 2 == 0:
                        rs_pair = nrm.tile([1, 2, QT], F32, tag="rsp")
                    qT_ap = qT_sb[:, h // 2, qslc]
                    av = psav.tile([65, QT], F32, name="av")
                    for pr in range(nkc // 2):
                        ps = pss.tile([128, 2, QT], F32, name="ps")
                        offs = [max(0, (2 * pr + j) * KC - qj * QT)
                                for j in range(2)]
                        for j in range(2):
                            kc = 2 * pr + j
                            o = offs[j]
                            nc.tensor.matmul(
                                ps[:, j, o:QT],
                                kTpad[:, h, kc * KC:(kc + 1) * KC],
                                qT_ap[:, o:QT],
                                start=True, stop=True)
                        pt = ptp.tile([128, 2, QT], F16, name="pt")
                        o0 = offs[0]
                        nc.scalar.activation(
                            pt[:, :, o0:QT], ps[:, :, o0:QT],
                            mybir.ActivationFunctionType.Exp,
                            scale=0.125)
                        for j in range(2):
                            kc = 2 * pr + j
                            if kc * KC >= qj * QT:
                                # mask only the 128-wide diagonal band
                                o = offs[j]
                                w = min(KC, QT - o)
                                nc.gpsimd.affine_select(
                                    out=pt[:, j, o:o + w],
                                    in_=pt[:, j, o:o + w],
                                    compare_op=mybir.AluOpType.is_ge,
                                    fill=0.0, base=qj * QT + o - kc * KC,
                                    pattern=[[1, w]],
                                    channel_multiplier=-1)
                        # filler PE work sits in the exp-latency window
                        pair_idx += 1
                        nf = ((len(fillers) * pair_idx) // npairs_total
                              - (len(fillers) * (pair_idx - 1)) // npairs_total)
                        for _ in range(nf):
                            fillers.pop(0)()
                        for j in range(2):
                            kc = 2 * pr + j
                            o = offs[j]
                            nc.tensor.matmul(av[:, o:QT],
                                             vaug[:, kc, h, :],
                                             pt[:, j, o:QT],
                                             start=(kc == 0),
                                             stop=(kc == nkc - 1))
                    po = 64 * (h % 2)
                    nc.vector.tensor_copy(
                        out=yTun[po:po + 64, qj * 2 + h // 2, :],
                        in_=av[0:64, :])
                    nc.vector.tensor_copy(
                        out=rs_pair[0:1, h % 2, :], in_=av[64:65, :])
                    if h % 2 == 1:
                        rt2s.append(recip_half(rs_pair))
                # any leftover fillers (shouldn't happen, but be safe)
                while fillers:
                    fillers.pop(0)()
                return rt2s

            # ---- fused pipeline ----
            # qkv(0) q/k: ec-outer on the (still idle) pss pair tiles so
            # the PE starts on the first-arriving DMA pieces
            psA = pss.tile([128, 2, QT], F32, name="ps")
            psB = pss.tile([128, 2, QT], F32, name="ps")
            for ec in range(EC):
                for rt in range(4):
                    tgt = psA if rt < 2 else psB
                    nc.tensor.matmul(tgt[:, rt % 2, :],
                                     wqk_sb[:, ec, rt * 128:(rt + 1) * 128],
                                     xT_sb[:, ec, 0:QT],
                                     start=(ec == 0), stop=(ec == EC - 1))
            nc.vector.tensor_copy(out=qT_sb[:, :, 0:QT], in_=psA[:, :, :])
            for pr2 in range(2):
                nc.vector.tensor_copy(out=kTpad[0:64, 2 * pr2, 0:QT],
                                      in_=psB[0:64, pr2, :])
                nc.vector.tensor_copy(out=kTpad[64:128, 2 * pr2 + 1, 0:QT],
                                      in_=psB[64:128, pr2, :])
            for t in qkv_thunks(0, qk_done=True):
                t()
            # fillers: attn qj <- qkv(qj+1); proj(0) -> attn1;
            # proj(1)+proj(2) -> attn3 (attn3 is ACT-limited and needs
            # the most PE filler); proj(3) -> tail
            rt2s = [None] * NQ
            rt2s[0] = attention(0, qkv_thunks(1))
            rt2s[1] = attention(1, qkv_thunks(2) + proj_thunks(0, rt2s[0]))
            rt2s[2] = attention(2, qkv_thunks(3))
            rt2s[3] = attention(3, proj_thunks(1, rt2s[1])
                                + proj_thunks(2, rt2s[2]))
            for t in proj_thunks(3, rt2s[3]):
                t()
    return nc


# ---------------------------------------------------------------- host

_NC_CACHE = []


def _get_nc():
    if not _NC_CACHE:
        _install_bir_fix()
        _NC_CACHE.append(build())
    return _NC_CACHE[0]


def make_in_maps(x, w_attn, w_proj):
    import ml_dtypes
    bf16 = ml_dtypes.bfloat16
    bmat = np.zeros((2, 128), np.float32)
    bmat[0, 0:64] = 1.0
    bmat[1, 64:128] = 1.0
    in_maps = []
    for c in range(N_CORES):
        b, h0 = c // 4, (c % 4) * HPC
        wq = w_attn[:, h0 * D:(h0 + HPC) * D]
        wk = w_attn[:, E + h0 * D:E + (h0 + HPC) * D]
        wv = w_attn[:, 2 * E + h0 * D:2 * E + (h0 + HPC) * D]
        in_maps.append({
            "xT": np.ascontiguousarray(x[b].T).astype(bf16),
            "wqk": np.ascontiguousarray(
                np.concatenate([wq, wk], axis=1)).astype(bf16),
            "wv": np.ascontiguousarray(wv).astype(bf16),
            "wproj": np.ascontiguousarray(
                w_proj[h0 * D:(h0 + HPC) * D, :]).astype(bf16),
            "bmat": bmat,
        })
    return in_maps


def run(x, w_attn, w_proj, trace=False, tmpdir=None):
    from concourse.bass_utils import run_bass_kernel_spmd
    nc = _get_nc()
    res = run_bass_kernel_spmd(nc, make_in_maps(x, w_attn, w_proj),
                               list(range(N_CORES)), trace=trace, tmpdir=tmpdir)
    y = np.zeros((B, S, E), np.float32)
    for c in range(N_CORES):
        y[c // 4] += np.asarray(res.results[c]["y"], np.float32)
    return y, res


def kernel(x, w_attn, w_proj):
    y, _ = run(np.asarray(x, np.float32), np.asarray(w_attn, np.float32),
               np.asarray(w_proj, np.float32))
    return y
